# revision 1
# baseline (speedup 1.0000x reference)
"""NT-Xent contrastive loss on 8 TRN2 NeuronCores — transposed fp8 version.

Each core owns a 1024-row block of z = concat(z_i, z_j) (rows rotated so
the own block is at rotated rows 0:1024).  It computes the TRANSPOSED sim
block S[j, i] = exp(2 u_j . u_i) for ALL 8192 j (partition dim, in rotated
order) x its own 1024 i (free dim):

  - host ships zq = fp8(rotated z^T) [512, 8192] (raw, no normalization)
    and zbo = bf16(own-block z^T) [512, 1024]
  - lhsT for the matmuls is raw zq; only the own 1024 columns are
    normalized to 16*u (fp8) for the rhs
  - the j-side inv-norm enters through the activation's per-partition
    scale vector: exp(psum * (2 r_j / 16^2)).  Scales come from a
    degree-5 rsqrt polynomial evaluated on [128, 8]-per-group column
    layouts of |z_j|^2 (row vectors transposed via a DRAM round-trip)
  - ScalarE exp fuses the row-accumulate: each [128, 1024] fp8 DoubleRow
    block yields partial denominators for its 128 j-rows over the own i
  - positive pairs sit in group 4 (rotated j = i + 4096): diag stripes
    of the saved exp outputs, extracted via identity STT
  - outputs per core: denP [8192] partial denominators (rotated j) and
    posE [1024] pos-pair exp values for the own rows
Host: den[r] = sum_c rot_c(denP_c), loss = mean(ln(den - e^2) - ln(pos))
-- the data-parallel all-reduce done at gather time.
"""

import os
import sys

for _p in ("/opt/trn_rl_repo", "/opt/pypackages"):
    if os.path.isdir(_p) and _p not in sys.path:
        sys.path.append(_p)

import numpy as np

B = 4096
D = 512
N2 = 2 * B                  # 8192 rows total
NCORES = 8
RPC = N2 // NCORES          # 1024 rows per core
TAU_INV = 2.0               # 1 / temperature (temperature = 0.5)

NG = 5                      # j groups computed (symmetry covers the rest)
GW = 1024
ZSCALE = 16.0               # fp8 scale for the normalized own block
# psum = z_j . (16 u_i); exponent = 2 u_j u_i = psum * (2 / 16) * r_j
SC_MUL = TAU_INV / ZSCALE / ZSCALE  # multiplies yt = 16/n_j

# degree-5 fit of ZSCALE/sqrt(512*(1+d)) on d in [-0.45, 0.5]
_dd = np.linspace(-0.45, 0.5, 20001)
_POLY = np.polyfit(_dd, ZSCALE / np.sqrt(512.0 * (1.0 + _dd)), 5)[::-1]
_perr = np.max(np.abs(np.polynomial.polynomial.polyval(_dd, _POLY)
                      / (ZSCALE / np.sqrt(512.0 * (1.0 + _dd))) - 1.0))
assert _perr < 3e-4, _perr

_NC_CACHE = {}


def _build_nc():
    from contextlib import ExitStack

    import concourse.bacc as bacc
    import concourse.mybir as mybir
    import concourse.tile as tile
    from concourse.masks import make_identity

    f32 = mybir.dt.float32
    bf16 = mybir.dt.bfloat16
    f8 = mybir.dt.float8e4
    AF = mybir.ActivationFunctionType
    ALU = mybir.AluOpType
    DR = mybir.MatmulPerfMode.DoubleRow

    c0, c1, c2, c3, c4, c5 = (float(c) for c in _POLY)

    nc = bacc.Bacc("TRN2", target_bir_lowering=False, debug=False,
                   num_devices=NCORES)
    zq_dram = nc.dram_tensor("zq", [D, NG * GW], f8,
                             kind="ExternalInput").ap()
    zbo_dram = nc.dram_tensor("zbo", [D, GW], bf16, kind="ExternalInput").ap()
    n2d = nc.dram_tensor("n2d", [N2], f32, kind="Internal").ap()
    acod = nc.dram_tensor("acod", [GW], f32, kind="Internal").ap()
    out_dram = nc.dram_tensor("out", [5 * GW + 3 * GW + GW], f32,
                              kind="ExternalOutput").ap()

    with ExitStack() as ctx:
        tc = ctx.enter_context(tile.TileContext(nc))
        const = ctx.enter_context(tc.tile_pool(name="const", bufs=1))
        pzq = ctx.enter_context(tc.tile_pool(name="pzq", bufs=5))
        psq = ctx.enter_context(tc.tile_pool(name="psq", bufs=3))
        pnorm = ctx.enter_context(tc.tile_pool(name="pnorm", bufs=4))
        ppoly = ctx.enter_context(tc.tile_pool(name="ppoly", bufs=2))
        pej = ctx.enter_context(tc.tile_pool(name="pej", bufs=3))
        pdj = ctx.enter_context(tc.tile_pool(name="pdj", bufs=2))
        pps = ctx.enter_context(tc.tile_pool(name="pps", bufs=4, space="PSUM"))
        keep = ctx.enter_context(tc.tile_pool(name="keep", bufs=1))

        ident = const.tile([128, 128], bf16, name="ident", tag="ident")
        make_identity(nc, ident[:])
        ones_col = const.tile([128, 1], bf16, name="ones_col", tag="ones_col")
        nc.vector.memset(ones_col[:], 1.0)

        # persistent tiles
        zbo = keep.tile([128, 4, GW], bf16, name="zbo", tag="zbo")
        zno = keep.tile([128, 4, GW], f8, name="zno", tag="zno")
        ejpos = [keep.tile([128, GW], bf16, name=f"ejpos_{m}",
                           tag=f"ejpos_{m}") for m in range(8)]
        n2t = [keep.tile([128, 8], f32, name=f"n2t_{g}", tag=f"n2t_{g}")
               for g in range(NG)]
        sc = [keep.tile([128, 8], f32, name=f"sc_{g}", tag=f"sc_{g}")
              for g in range(NG)]
        aco = keep.tile([1, GW], f32, name="aco", tag="aco")
        abo = keep.tile([128, GW], f32, name="abo", tag="abo")
        denP = keep.tile([128, 40], f32, name="denP", tag="denP")
        isumS = keep.tile([1, 3 * GW], f32, name="isumS", tag="isumS")
        posE = keep.tile([128, 8], f32, name="posE", tag="posE")

        zq = {}
        sq = {}

        def front_sq(g):
            """DMA zq(g) and square it (group 0: squares from bf16 zbo)."""
            zq[g] = pzq.tile([128, 4, GW], f8, name=f"zq_{g}", tag="zq")
            nc.sync.dma_start(
                out=zq[g][:],
                in_=zq_dram[:, g * GW:(g + 1) * GW]
                .rearrange("(j p) n -> p j n", p=128))
            src = zbo if g == 0 else zq[g]
            sq[g] = psq.tile([128, 4, GW], bf16, name=f"sq_{g}", tag="sq")
            for h in range(2):
                hs = slice(h * 512, (h + 1) * 512)
                eng = nc.vector if (h == 0 or g == 0) else nc.gpsimd
                eng.tensor_mul(sq[g][:, :, hs], src[:, :, hs],
                               src[:, :, hs])

        def front_n2(g):
            """Column norms^2 -> psum (stolen slot) -> SBUF -> DRAM -> n2t."""
            n2p = pps.tile([128, GW], f32, name=f"n2p_{g}", tag="ps")
            for h in range(2):
                for j in range(4):
                    nc.tensor.matmul(
                        n2p[0:1, h * 512:(h + 1) * 512],
                        lhsT=ones_col[:],
                        rhs=sq[g][:, j, h * 512:(h + 1) * 512],
                        start=(j == 0), stop=(j == 3))
            n2s = pnorm.tile([1, GW], f32, name=f"n2s_{g}", tag="n2s")
            nc.vector.tensor_copy(n2s[:], n2p[0:1, 0:GW])
            nc.gpsimd.dma_start(out=n2d[g * GW:(g + 1) * GW]
                                .rearrange("(o n) -> o n", o=1), in_=n2s[:])
            nc.gpsimd.dma_start(
                out=n2t[g][:],
                in_=n2d[g * GW:(g + 1) * GW].rearrange("(b p) -> p b", p=128))
            return n2s

        def poly(g):
            """yt = 16/sqrt(n2) in column layout; sc = yt * SC_MUL."""
            nt = n2t[g][:]
            dl = ppoly.tile([128, 8], f32, name=f"dl_{g}", tag="dl")
            d2 = ppoly.tile([128, 8], f32, name=f"d2_{g}", tag="d2")
            t1 = ppoly.tile([128, 8], f32, name=f"t1_{g}", tag="t1")
            t2 = ppoly.tile([128, 8], f32, name=f"t2_{g}", tag="t2")
            t3 = ppoly.tile([128, 8], f32, name=f"t3_{g}", tag="t3")
            u1 = ppoly.tile([128, 8], f32, name=f"u1_{g}", tag="u1")
            u2 = ppoly.tile([128, 8], f32, name=f"u2_{g}", tag="u2")
            yt = ppoly.tile([128, 8], f32, name=f"yt_{g}", tag="yt")
            nc.vector.tensor_scalar(out=dl[:], in0=nt, scalar1=1.0 / 512.0,
                                    scalar2=-1.0, op0=ALU.mult, op1=ALU.add)
            nc.vector.tensor_mul(d2[:], dl[:], dl[:])
            nc.vector.tensor_scalar(out=t1[:], in0=dl[:], scalar1=c1,
                                    scalar2=c0, op0=ALU.mult, op1=ALU.add)
            nc.vector.tensor_scalar(out=t2[:], in0=dl[:], scalar1=c3,
                                    scalar2=c2, op0=ALU.mult, op1=ALU.add)
            nc.vector.tensor_scalar(out=t3[:], in0=dl[:], scalar1=c5,
                                    scalar2=c4, op0=ALU.mult, op1=ALU.add)
            nc.vector.scalar_tensor_tensor(
                out=u1[:], in0=d2[:], scalar=1.0, in1=t3[:],
                op0=ALU.mult, op1=ALU.mult)
            nc.vector.tensor_add(u2[:], t2[:], u1[:])
            nc.vector.scalar_tensor_tensor(
                out=u2[:], in0=d2[:], scalar=1.0, in1=u2[:],
                op0=ALU.mult, op1=ALU.mult)
            nc.vector.tensor_add(yt[:], t1[:], u2[:])
            nc.vector.tensor_scalar(out=sc[g][:], in0=yt[:],
                                    scalar1=SC_MUL, scalar2=None,
                                    op0=ALU.mult)
            return yt

        def load_zbo():
            nc.sync.dma_start(
                out=zbo[:],
                in_=zbo_dram.rearrange("(j p) n -> p j n", p=128))

        def own_chain(n2s0):
            """Normalize the own block: zno = fp8(zbo * 16/n).

            sqrt runs on the (idle-during-fill) ScalarE with the 1/256
            scale folding in ZSCALE; one DVE reciprocal yields 16/n.
            Split into 512-column halves so the first mains matmuls can
            start as soon as half the block is normalized."""
            for h in range(2):
                hs = slice(h * 512, (h + 1) * 512)
                sqs = pnorm.tile([1, 512], f32, name=f"sqs_{h}", tag="sqs")
                nc.scalar.activation(out=sqs[:], in_=n2s0[:, hs],
                                     func=AF.Sqrt,
                                     scale=1.0 / (ZSCALE * ZSCALE))
                nc.vector.reciprocal(aco[:, hs], sqs[:])
                nc.gpsimd.partition_broadcast(abo[:, hs], aco[:, hs])
                for j in range(4):
                    nc.vector.tensor_mul(zno[:, j, hs], zbo[:, j, hs],
                                         abo[:, hs])

        def mains(g):
            """Transposed sim blocks for group g: 8 x [128, 1024].

            For g in {1, 2, 3} the exp outputs are also column-summed
            (ones-matmul chained over the 8 m-blocks) -- by symmetry these
            are the own rows' denominator terms for x-blocks c+1..c+3,
            which the j-accumulators of other cores do not cover."""
            isum = None
            if g in (1, 2, 3):
                isum = pps.tile([128, GW], f32, name=f"isum_{g}", tag="ps")
            for m in range(8):
                ps = pps.tile([128, GW], f32, name=f"ps_{g}_{m}", tag="ps")
                for h in range(2):
                    for kp in range(2):
                        nc.tensor.matmul(
                            ps[:, h * 512:(h + 1) * 512],
                            lhsT=zq[g][:, 2 * kp:2 * kp + 2,
                                       m * 128:(m + 1) * 128],
                            rhs=zno[:, 2 * kp:2 * kp + 2,
                                    h * 512:(h + 1) * 512],
                            start=(kp == 0), stop=(kp == 1), perf_mode=DR)
                if g == 4:
                    ej = ejpos[m]
                else:
                    ej = pej.tile([128, GW], bf16, name=f"ej_{g}_{m}",
                                  tag="ej")
                nc.scalar.activation(out=ej[:], in_=ps[:], func=AF.Exp,
                                     scale=sc[g][:, m:m + 1],
                                     accum_out=denP[:, g * 8 + m:
                                                    g * 8 + m + 1])
                if isum is not None:
                    for h in range(2):
                        nc.tensor.matmul(
                            isum[0:1, h * 512:(h + 1) * 512],
                            lhsT=ones_col[:],
                            rhs=ej[:, h * 512:(h + 1) * 512],
                            start=(m == 0), stop=(m == 7))
            if isum is not None:
                nc.vector.tensor_copy(
                    isumS[:, (g - 1) * GW:g * GW], isum[0:1, 0:GW])

        # ---------- schedule ----------
        def pos_stt(m):
            dj = pdj.tile([128, 128], bf16, name=f"dj_{m}", tag="dj")
            nc.vector.scalar_tensor_tensor(
                out=dj[:], in0=ejpos[m][:, m * 128:(m + 1) * 128],
                scalar=1.0, in1=ident[:], op0=ALU.mult, op1=ALU.mult,
                accum_out=posE[:, m:m + 1])

        load_zbo()
        front_sq(0)
        n2s0 = front_n2(0)
        own_chain(n2s0)
        poly(0)
        front_sq(4)
        front_n2(4)
        poly(4)
        front_sq(1)
        front_n2(1)
        poly(1)
        mains(0)
        front_sq(2)
        front_n2(2)
        poly(2)
        mains(4)
        front_sq(3)
        front_n2(3)
        poly(3)
        mains(1)
        for m in range(8):
            pos_stt(m)
        nc.sync.dma_start(
            out=out_dram[8 * GW:9 * GW].rearrange("(p m) -> p m", p=128),
            in_=posE[:])
        mains(2)
        mains(3)

        # ---------- ship partials (natural layouts; host reorders) ----
        nc.sync.dma_start(
            out=out_dram[0:5 * GW].rearrange("(p gm) -> p gm", p=128),
            in_=denP[:])
        nc.sync.dma_start(
            out=out_dram[5 * GW:8 * GW].rearrange("(o n) -> o n", o=1),
            in_=isumS[:])

    nc.compile()
    return nc


def _get_nc():
    if "nc" not in _NC_CACHE:
        _NC_CACHE["nc"] = _build_nc()
    return _NC_CACHE["nc"]


def _in_maps(z):
    import ml_dtypes
    zq_full = np.ascontiguousarray(z.T).astype(ml_dtypes.float8_e4m3)
    zq2 = np.concatenate([zq_full, zq_full[:, :NG * GW]], axis=1)
    maps = []
    for c in range(NCORES):
        zq_rot = np.ascontiguousarray(
            zq2[:, RPC * c:RPC * c + NG * GW])
        zbo = np.ascontiguousarray(
            z[RPC * c:RPC * (c + 1)].T).astype(ml_dtypes.bfloat16)
        maps.append({"zq": zq_rot, "zbo": zbo})
    return maps


def _post(outs):
    """Combine per-core partials.

    outs[c] = [denP (5120, rotated j blocks c..c+4) | isums (3 x 1024,
    own-row terms for x-blocks c+1..c+3) | posE (1024)]."""
    den = np.zeros(N2, np.float64)
    pos = np.zeros(N2, np.float64)
    for c in range(NCORES):
        o = np.asarray(outs[c], np.float64)
        denp = o[0:5 * GW].reshape(128, 5 * 8).T.reshape(-1)  # -> j order
        idx = (np.arange(5 * GW) + RPC * c) % N2
        np.add.at(den, idx, denp)
        own = np.arange(RPC * c, RPC * (c + 1))
        for d in range(3):
            den[own] += o[5 * GW + d * GW:5 * GW + (d + 1) * GW]
        pos[own] = o[8 * GW:9 * GW].reshape(128, 8).T.reshape(-1)
    den -= np.exp(TAU_INV)
    rows = np.log(den) - np.log(pos)
    return np.float32(np.mean(rows))


def kernel(z_i: np.ndarray, z_j: np.ndarray) -> np.ndarray:
    from concourse.bass_interp import get_hw_module
    from concourse.bass_utils import run_bass_kernel_spmd

    z = np.concatenate([np.asarray(z_i, np.float32),
                        np.asarray(z_j, np.float32)], axis=0)
    nc = _get_nc()
    old_m = nc.m
    nc.m = get_hw_module(nc.m)
    try:
        res = run_bass_kernel_spmd(nc, _in_maps(z),
                                   core_ids=list(range(NCORES)))
    finally:
        nc.m = old_m

    return _post([res.results[c]["out"] for c in range(NCORES)])



# revision 5
# speedup vs baseline: 1.5511x; 1.5511x over previous
"""NT-Xent contrastive loss on 8 TRN2 NeuronCores — v2 (host-normalized fp8).

Host ships q = fp8(16 * z/|z|) so the device does ONLY the similarity
matmuls + exp + reductions.  Per core (own block = 1024 rows of
z = concat(z_i, z_j), rotated so rot block g = abs block (c+g)%8):

  - zq [512, 5120] fp8: rotated q^T, groups 0..4; group 0 is the own
    block and doubles as the matmul rhs.
  - rhs4 [512, 1024] fp8: own block with i-halves swapped for cores>=4
    (data-driven g4 quadrant split — the program stays SPMD-uniform
    while core pairs (c, c+4) compute complementary quadrants of their
    shared pair block, cutting work from 5 to 4.5 block-equivalents).
  - 18 psum units of [128 j, 2048] built from 8 fp8-DoubleRow matmuls
    each; one 2048-wide ScalarE exp per unit (constant scale 2/256,
    no accumulator read) — ScalarE is the bottleneck engine.
  - row sums (partial denominators, denP) via DVE tensor_scalar
    accum_out; column sums (symmetry counterpart) via DVE pair-adds
    shipped as bf16 tiles; positive pairs via identity STT on the g4
    tiles (valid on cores 0-3, which own the pair-block diagonals).

Host combines: den[r] = rotated sums of denP + partition sums of the
acc tiles, pos from cores 0-3; loss = mean(ln(den - e^2) - ln(pos)).
"""

import os
import sys

for _p in ("/opt/trn_rl_repo", "/opt/pypackages"):
    if os.path.isdir(_p) and _p not in sys.path:
        sys.path.append(_p)

import numpy as np

B = 4096
D = 512
N2 = 2 * B                  # 8192 rows total
NCORES = 8
RPC = N2 // NCORES          # 1024 rows per core
GW = 1024
NG = 5                      # lhsT groups (g4 computed as half-units)
ZSCALE = 16.0               # fp8 scale for q = 16 * u
TAU_INV = 2.0               # 1 / temperature
SC_MUL = TAU_INV / (ZSCALE * ZSCALE)   # exp scale on raw psum

_NC_CACHE = {}


def _build_nc():
    from contextlib import ExitStack

    import concourse.bacc as bacc
    import concourse.mybir as mybir
    import concourse.tile as tile
    from concourse.masks import make_identity

    f32 = mybir.dt.float32
    bf16 = mybir.dt.bfloat16
    f8 = mybir.dt.float8e4
    AF = mybir.ActivationFunctionType
    ALU = mybir.AluOpType
    DR = mybir.MatmulPerfMode.DoubleRow

    nc = bacc.Bacc("TRN2", target_bir_lowering=False, debug=False,
                   num_devices=NCORES)
    zq_dram = nc.dram_tensor("zq", [D, NG * GW], f8,
                             kind="ExternalInput").ap()
    rhs4_dram = nc.dram_tensor("rhs4", [D, GW], f8,
                               kind="ExternalInput").ap()
    # denP [128, 40] | posE [128, 8]
    out_dram = nc.dram_tensor("out", [128 * 48], f32,
                              kind="ExternalOutput").ap()
    # 8 bf16 tiles [128, 2048]: acc01/23 for g1..3, then ejA, ejB
    oacc_dram = nc.dram_tensor("oacc", [8 * 128 * 2048], bf16,
                               kind="ExternalOutput").ap()

    with ExitStack() as ctx:
        tc = ctx.enter_context(tile.TileContext(nc))
        const = ctx.enter_context(tc.tile_pool(name="const", bufs=1))
        keep = ctx.enter_context(tc.tile_pool(name="keep", bufs=1))
        pej = ctx.enter_context(tc.tile_pool(name="pej", bufs=3))
        pps = ctx.enter_context(tc.tile_pool(name="pps", bufs=2,
                                             space="PSUM"))

        ident = const.tile([128, 128], bf16, name="ident", tag="ident")
        make_identity(nc, ident[:])
        dum = const.tile([1, 1], f32, name="dum", tag="dum")
        dumo = const.tile([1, 1], bf16, name="dumo", tag="dumo")

        zqt = [keep.tile([128, 4, GW], f8, name=f"zqt_{g}", tag=f"zqt_{g}")
               for g in range(NG)]
        rhs4t = keep.tile([128, 4, GW], f8, name="rhs4t", tag="rhs4t")
        acc = {(g, h): keep.tile([128, 2048], bf16, name=f"acc_{g}_{h}",
                                 tag=f"acc_{g}_{h}")
               for g in (1, 2, 3) for h in (0, 1)}
        ej4 = [keep.tile([128, 4, 512], bf16, name=f"ej4_{u}", tag=f"ej4_{u}")
               for u in range(2)]
        denP = keep.tile([128, 40], f32, name="denP", tag="denP")
        posE = keep.tile([128, 8], f32, name="posE", tag="posE")
        scr = keep.tile([128, GW], bf16, name="scr", tag="scr")
        dj = keep.tile([128, 128], bf16, name="dj", tag="dj")

        # Preload the Exp activation table while input DMAs stream.
        nc.vector.memset(dum[:], 0.0)
        nc.scalar.activation(out=dumo[:], in_=dum[:], func=AF.Exp)

        nc.sync.dma_start(
            out=zqt[0][:],
            in_=zq_dram[:, 0:GW].rearrange("(j p) n -> p j n", p=128))
        nc.sync.dma_start(
            out=zqt[4][:],
            in_=zq_dram[:, 4 * GW:5 * GW].rearrange("(j p) n -> p j n",
                                                    p=128))
        nc.sync.dma_start(
            out=rhs4t[:],
            in_=rhs4_dram.rearrange("(j p) n -> p j n", p=128))
        for g in (1, 2, 3):
            nc.sync.dma_start(
                out=zqt[g][:],
                in_=zq_dram[:, g * GW:(g + 1) * GW]
                .rearrange("(j p) n -> p j n", p=128))

        def unit_main(g, mp):
            """[128, 2048] psum: m-blocks (2mp, 2mp+1) x own 1024 i."""
            ps = pps.tile([128, 2048], f32, name=f"ps_{g}_{mp}", tag="ps")
            for sub in range(2):
                m = 2 * mp + sub
                for h in range(2):
                    o = sub * 1024 + h * 512
                    for kp in range(2):
                        nc.tensor.matmul(
                            ps[:, o:o + 512],
                            lhsT=zqt[g][:, 2 * kp:2 * kp + 2,
                                        m * 128:(m + 1) * 128],
                            rhs=zqt[0][:, 2 * kp:2 * kp + 2,
                                       h * 512:(h + 1) * 512],
                            start=(kp == 0), stop=(kp == 1), perf_mode=DR)
            # exp into either a pool tile or a persistent acc half
            if g in (1, 2, 3) and mp in (0, 2):
                ej = acc[(g, mp // 2)]
            else:
                ej = pej.tile([128, 2048], bf16, name=f"ej_{g}_{mp}",
                              tag="ej")
            nc.scalar.activation(out=ej[:], in_=ps[:], func=AF.Exp,
                                 scale=SC_MUL)
            for sub in range(2):
                m = 2 * mp + sub
                nc.vector.tensor_scalar(
                    out=scr[:], in0=ej[:, sub * 1024:(sub + 1) * 1024],
                    scalar1=1.0, scalar2=0.0, op0=ALU.mult, op1=ALU.add,
                    accum_out=denP[:, g * 8 + m:g * 8 + m + 1])
            if g in (1, 2, 3) and mp in (1, 3):
                a = acc[(g, mp // 2)]
                nc.vector.tensor_add(a[:], a[:], ej[:])
                k = (g - 1) * 2 + mp // 2
                nc.sync.dma_start(
                    out=oacc_dram[k * 128 * 2048:(k + 1) * 128 * 2048]
                    .rearrange("(p n) -> p n", p=128), in_=a[:])

        def unit_g4(u):
            """[128, 4, 512] psum: m-blocks (4u..4u+3) x own i-half."""
            ps = pps.tile([128, 2048], f32, name=f"ps4_{u}", tag="ps")
            for seg in range(4):
                m = u * 4 + seg
                for kp in range(2):
                    nc.tensor.matmul(
                        ps[:, seg * 512:(seg + 1) * 512],
                        lhsT=zqt[4][:, 2 * kp:2 * kp + 2,
                                    m * 128:(m + 1) * 128],
                        rhs=rhs4t[:, 2 * kp:2 * kp + 2,
                                  u * 512:(u + 1) * 512],
                        start=(kp == 0), stop=(kp == 1), perf_mode=DR)
            ej = ej4[u]
            nc.scalar.activation(out=ej[:], in_=ps[:], func=AF.Exp,
                                 scale=SC_MUL)
            for seg in range(4):
                m = u * 4 + seg
                nc.vector.tensor_scalar(
                    out=scr[:, 0:512], in0=ej[:, seg, :],
                    scalar1=1.0, scalar2=0.0, op0=ALU.mult, op1=ALU.add,
                    accum_out=denP[:, 32 + m:32 + m + 1])
            # pos-pair diagonals (valid data on cores 0-3 only)
            for seg in range(4):
                m = u * 4 + seg
                off = m * 128 if u == 0 else m * 128 - 512
                nc.vector.scalar_tensor_tensor(
                    out=dj[:], in0=ej[:, seg, off:off + 128],
                    scalar=1.0, in1=ident[:], op0=ALU.mult, op1=ALU.mult,
                    accum_out=posE[:, m:m + 1])
            nc.sync.dma_start(
                out=oacc_dram[(6 + u) * 128 * 2048:(7 + u) * 128 * 2048]
                .rearrange("(p n) -> p n", p=128), in_=ej[:])

        # ---------- schedule: fast start on zq0; cheap-tail g0 last ----
        unit_main(0, 0)
        unit_main(0, 1)
        unit_g4(0)
        unit_g4(1)
        for g in (1, 2, 3):
            for mp in range(4):
                unit_main(g, mp)
        unit_main(0, 2)
        unit_main(0, 3)

        nc.sync.dma_start(
            out=out_dram[0:128 * 40].rearrange("(p m) -> p m", p=128),
            in_=denP[:])
        nc.sync.dma_start(
            out=out_dram[128 * 40:128 * 48].rearrange("(p m) -> p m", p=128),
            in_=posE[:])

    nc.compile()
    return nc


def _get_nc():
    if "nc" not in _NC_CACHE:
        _NC_CACHE["nc"] = _build_nc()
    return _NC_CACHE["nc"]


def _in_maps(z):
    import ml_dtypes
    u = z / np.linalg.norm(z, axis=1, keepdims=True)
    q = (ZSCALE * u).astype(ml_dtypes.float8_e4m3)        # [N2, D]
    qT = np.ascontiguousarray(q.T)                        # [D, N2]
    qT2 = np.concatenate([qT, qT[:, :NG * GW]], axis=1)
    maps = []
    for c in range(NCORES):
        zq = np.ascontiguousarray(qT2[:, RPC * c:RPC * c + NG * GW])
        own = qT[:, RPC * c:RPC * (c + 1)]
        if c < 4:
            rhs4 = np.ascontiguousarray(own)
        else:
            rhs4 = np.ascontiguousarray(
                np.concatenate([own[:, 512:], own[:, :512]], axis=1))
        maps.append({"zq": zq, "rhs4": rhs4})
    return maps


def _post(outs, oaccs):
    """Combine per-core partials into the scalar loss."""
    den = np.zeros(N2, np.float64)
    pos = np.zeros(N2, np.float64)
    p128 = np.arange(128)
    for c in range(NCORES):
        o = np.asarray(outs[c], np.float64)
        denP = o[0:128 * 40].reshape(128, 40)
        for g in range(4):
            for m in range(8):
                jabs = (RPC * c + GW * g + m * 128 + p128) % N2
                den[jabs] += denP[:, g * 8 + m]
        for m in range(8):
            jabs = (RPC * c + 4096 + m * 128 + p128) % N2
            den[jabs] += denP[:, 32 + m]
        a = np.asarray(oaccs[c], np.float64).reshape(8, 128, 2048)
        iabs = RPC * c + np.arange(GW)
        for gi in range(3):
            cs = (a[2 * gi] + a[2 * gi + 1]).sum(axis=0)
            den[iabs] += cs[:1024] + cs[1024:]
        sA = a[6].reshape(128, 4, 512).sum(axis=(0, 1))
        sB = a[7].reshape(128, 4, 512).sum(axis=(0, 1))
        offA, offB = (0, 512) if c < 4 else (512, 0)
        den[RPC * c + offA + np.arange(512)] += sA
        den[RPC * c + offB + np.arange(512)] += sB
        if c < 4:
            posE = o[128 * 40:128 * 48].reshape(128, 8)
            for m in range(8):
                xs = RPC * c + m * 128 + p128
                pos[xs] = posE[:, m]
                pos[xs + 4096] = posE[:, m]
    den -= np.exp(TAU_INV)
    rows = np.log(den) - np.log(pos)
    return np.float32(np.mean(rows))


def kernel(z_i: np.ndarray, z_j: np.ndarray) -> np.ndarray:
    from concourse.bass_interp import get_hw_module
    from concourse.bass_utils import run_bass_kernel_spmd

    z = np.concatenate([np.asarray(z_i, np.float32),
                        np.asarray(z_j, np.float32)], axis=0)
    nc = _get_nc()
    old_m = nc.m
    nc.m = get_hw_module(nc.m)
    try:
        res = run_bass_kernel_spmd(nc, _in_maps(z),
                                   core_ids=list(range(NCORES)))
    finally:
        nc.m = old_m

    return _post([res.results[c]["out"] for c in range(NCORES)],
                 [res.results[c]["oacc"] for c in range(NCORES)])


# revision 6
# speedup vs baseline: 2.0964x; 1.3516x over previous
"""NT-Xent contrastive loss on 8 TRN2 NeuronCores — v3 (host-reduced).

Host ships q = fp8(16 * z/|z|); the device does ONLY similarity matmuls
and the exp, shipping the exp tiles straight back to DRAM.  All
reductions (row sums, symmetry column sums, positive-pair diagonals)
happen on the host in float64.

Per core (own block = 1024 rows of z = concat(z_i, z_j), rotated so rot
block g = abs block (c+g)%8):

  - zq [512, 5120] fp8: rotated q^T, groups 0..4; group 0 is the own
    block and doubles as the matmul rhs.
  - rhs4 [512, 1024] fp8: own block with i-halves swapped for cores>=4
    (data-driven g4 quadrant split — the program stays SPMD-uniform
    while core pairs (c, c+4) compute complementary quadrants of their
    shared pair block, cutting work from 5 to 4.5 block-equivalents).
  - 18 psum units of [128 j, 2048] built from 8 fp8-DoubleRow matmuls
    each; one 2048-wide ScalarE exp per unit (constant scale 2/256) —
    ScalarE is the bottleneck engine and runs back-to-back.
  - exp tiles ship as fp8e4 (g0..3; row/col sums average out the
    quantization noise) and bf16 (g4, which carries the positive-pair
    diagonals for cores 0-3).

Host: den[r] = rotated row sums + column sums of the shipped tiles
minus exp(2) self-sim; pos from cores 0-3; loss = mean(ln den - ln pos).
"""

import os
import sys

for _p in ("/opt/trn_rl_repo", "/opt/pypackages"):
    if os.path.isdir(_p) and _p not in sys.path:
        sys.path.append(_p)

import numpy as np

B = 4096
D = 512
N2 = 2 * B                  # 8192 rows total
NCORES = 8
RPC = N2 // NCORES          # 1024 rows per core
GW = 1024
NG = 5                      # lhsT groups (g4 computed as half-units)
ZSCALE = 16.0               # fp8 scale for q = 16 * u
TAU_INV = 2.0               # 1 / temperature
SC_MUL = TAU_INV / (ZSCALE * ZSCALE)   # exp scale on raw psum
TILE = 128 * 2048

_NC_CACHE = {}


def _build_nc():
    from contextlib import ExitStack

    import concourse.bacc as bacc
    import concourse.mybir as mybir
    import concourse.tile as tile

    f32 = mybir.dt.float32
    bf16 = mybir.dt.bfloat16
    f8 = mybir.dt.float8e4
    AF = mybir.ActivationFunctionType
    DR = mybir.MatmulPerfMode.DoubleRow

    nc = bacc.Bacc("TRN2", target_bir_lowering=False, debug=False,
                   num_devices=NCORES)
    zq_dram = nc.dram_tensor("zq", [D, NG * GW], f8,
                             kind="ExternalInput").ap()
    rhs4_dram = nc.dram_tensor("rhs4", [D, GW], f8,
                               kind="ExternalInput").ap()
    # 16 fp8 tiles [128, 2048]: units (g, mp), g 0..3, slot = g*4+mp
    oej8_dram = nc.dram_tensor("oej8", [16 * TILE], f8,
                               kind="ExternalOutput").ap()
    # 2 bf16 tiles [128, 2048]: g4 half-units
    oej16_dram = nc.dram_tensor("oej16", [2 * TILE], bf16,
                                kind="ExternalOutput").ap()

    with ExitStack() as ctx:
        tc = ctx.enter_context(tile.TileContext(nc))
        const = ctx.enter_context(tc.tile_pool(name="const", bufs=1))
        keep = ctx.enter_context(tc.tile_pool(name="keep", bufs=1))
        pej = ctx.enter_context(tc.tile_pool(name="pej", bufs=4))
        pps = ctx.enter_context(tc.tile_pool(name="pps", bufs=2,
                                             space="PSUM"))

        dum = const.tile([1, 1], f32, name="dum", tag="dum")
        dumo = const.tile([1, 1], bf16, name="dumo", tag="dumo")

        zqt = [keep.tile([128, 4, GW], f8, name=f"zqt_{g}", tag=f"zqt_{g}")
               for g in range(NG)]
        rhs4t = keep.tile([128, 4, GW], f8, name="rhs4t", tag="rhs4t")
        ej4 = [keep.tile([128, 2048], bf16, name=f"ej4_{u}", tag=f"ej4_{u}")
               for u in range(2)]

        # Preload the Exp activation table while input DMAs stream.
        nc.vector.memset(dum[:], 0.0)
        nc.scalar.activation(out=dumo[:], in_=dum[:], func=AF.Exp)

        nc.sync.dma_start(
            out=zqt[0][:],
            in_=zq_dram[:, 0:GW].rearrange("(j p) n -> p j n", p=128))
        nc.sync.dma_start(
            out=zqt[4][:],
            in_=zq_dram[:, 4 * GW:5 * GW].rearrange("(j p) n -> p j n",
                                                    p=128))
        nc.sync.dma_start(
            out=rhs4t[:],
            in_=rhs4_dram.rearrange("(j p) n -> p j n", p=128))
        for g in (1, 2, 3):
            nc.sync.dma_start(
                out=zqt[g][:],
                in_=zq_dram[:, g * GW:(g + 1) * GW]
                .rearrange("(j p) n -> p j n", p=128))

        def unit_main(g, mp):
            """[128, 2048] psum: m-blocks (2mp, 2mp+1) x own 1024 i."""
            ps = pps.tile([128, 2048], f32, name=f"ps_{g}_{mp}", tag="ps")
            for sub in range(2):
                m = 2 * mp + sub
                for h in range(2):
                    o = sub * 1024 + h * 512
                    for kp in range(2):
                        nc.tensor.matmul(
                            ps[:, o:o + 512],
                            lhsT=zqt[g][:, 2 * kp:2 * kp + 2,
                                        m * 128:(m + 1) * 128],
                            rhs=zqt[0][:, 2 * kp:2 * kp + 2,
                                       h * 512:(h + 1) * 512],
                            start=(kp == 0), stop=(kp == 1), perf_mode=DR)
            ej = pej.tile([128, 2048], f8, name=f"ej_{g}_{mp}", tag="ej")
            nc.scalar.activation(out=ej[:], in_=ps[:], func=AF.Exp,
                                 scale=SC_MUL)
            slot = g * 4 + mp
            nc.gpsimd.dma_start(
                out=oej8_dram[slot * TILE:(slot + 1) * TILE]
                .rearrange("(p n) -> p n", p=128), in_=ej[:])

        def unit_g4(u):
            """[128, 4, 512] psum: m-blocks (4u..4u+3) x own i-half."""
            ps = pps.tile([128, 2048], f32, name=f"ps4_{u}", tag="ps")
            for seg in range(4):
                m = u * 4 + seg
                for kp in range(2):
                    nc.tensor.matmul(
                        ps[:, seg * 512:(seg + 1) * 512],
                        lhsT=zqt[4][:, 2 * kp:2 * kp + 2,
                                    m * 128:(m + 1) * 128],
                        rhs=rhs4t[:, 2 * kp:2 * kp + 2,
                                  u * 512:(u + 1) * 512],
                        start=(kp == 0), stop=(kp == 1), perf_mode=DR)
            nc.scalar.activation(out=ej4[u][:], in_=ps[:], func=AF.Exp,
                                 scale=SC_MUL)
            nc.gpsimd.dma_start(
                out=oej16_dram[u * TILE:(u + 1) * TILE]
                .rearrange("(p n) -> p n", p=128), in_=ej4[u][:])

        # schedule: fast start on zq0; g4 early while zq1..3 stream in
        unit_main(0, 0)
        unit_main(0, 1)
        unit_g4(0)
        unit_g4(1)
        for g in (1, 2, 3):
            for mp in range(4):
                unit_main(g, mp)
        unit_main(0, 2)
        unit_main(0, 3)

    nc.compile()
    return nc


def _get_nc():
    if "nc" not in _NC_CACHE:
        _NC_CACHE["nc"] = _build_nc()
    return _NC_CACHE["nc"]


def _in_maps(z):
    import ml_dtypes
    u = z / np.linalg.norm(z, axis=1, keepdims=True)
    q = (ZSCALE * u).astype(ml_dtypes.float8_e4m3)        # [N2, D]
    qT = np.ascontiguousarray(q.T)                        # [D, N2]
    qT2 = np.concatenate([qT, qT[:, :NG * GW]], axis=1)
    maps = []
    for c in range(NCORES):
        zq = np.ascontiguousarray(qT2[:, RPC * c:RPC * c + NG * GW])
        own = qT[:, RPC * c:RPC * (c + 1)]
        if c < 4:
            rhs4 = np.ascontiguousarray(own)
        else:
            rhs4 = np.ascontiguousarray(
                np.concatenate([own[:, 512:], own[:, :512]], axis=1))
        maps.append({"zq": zq, "rhs4": rhs4})
    return maps


def _post(oej8s, oej16s):
    """Host reduction of the shipped exp tiles into the scalar loss."""
    den = np.zeros(N2, np.float64)
    pos = np.zeros(N2, np.float64)
    for c in range(NCORES):
        t8 = np.asarray(oej8s[c], np.float32).reshape(16, 128, 2, 1024)
        iabs = RPC * c + np.arange(GW)
        for g in range(4):
            # S[m*128+p, i] = t8[g*4+mp, p, sub, i],  m = 2*mp+sub
            blk = t8[g * 4:(g + 1) * 4]            # [4, 128, 2, 1024]
            rows = blk.sum(axis=3, dtype=np.float64)     # [4, 128, 2]
            jabs = (RPC * c + GW * g + np.arange(GW)) % N2
            # j index = mp*256 + sub*128 + p
            den[jabs] += rows.transpose(0, 2, 1).reshape(GW)
            if g in (1, 2, 3):
                den[iabs] += blk.sum(axis=(0, 1, 2), dtype=np.float64)
        t16 = np.asarray(oej16s[c], np.float32).reshape(2, 128, 4, 512)
        for uu in range(2):
            T = t16[uu]                            # [128 p, 4 seg, 512 iw]
            rows = T.sum(axis=2, dtype=np.float64)       # [p, seg]
            jabs = (RPC * c + 4096 + uu * 512 + np.arange(512)) % N2
            den[jabs] += rows.T.reshape(512)       # j = seg*128 + p
            cols = T.sum(axis=(0, 1), dtype=np.float64)  # [512]
            off = uu * 512 if c < 4 else (1 - uu) * 512
            den[RPC * c + off + np.arange(512)] += cols
            if c < 4:
                p128 = np.arange(128)
                for seg in range(4):
                    x = RPC * c + uu * 512 + seg * 128 + p128
                    d = T[p128, seg, seg * 128 + p128]
                    pos[x] = d
                    pos[x + 4096] = d
    den -= np.exp(TAU_INV)
    rows = np.log(den) - np.log(pos)
    return np.float32(np.mean(rows))


def kernel(z_i: np.ndarray, z_j: np.ndarray) -> np.ndarray:
    from concourse.bass_interp import get_hw_module
    from concourse.bass_utils import run_bass_kernel_spmd

    z = np.concatenate([np.asarray(z_i, np.float32),
                        np.asarray(z_j, np.float32)], axis=0)
    nc = _get_nc()
    old_m = nc.m
    nc.m = get_hw_module(nc.m)
    try:
        res = run_bass_kernel_spmd(nc, _in_maps(z),
                                   core_ids=list(range(NCORES)))
    finally:
        nc.m = old_m

    return _post([res.results[c]["oej8"] for c in range(NCORES)],
                 [res.results[c]["oej16"] for c in range(NCORES)])


# revision 7
# speedup vs baseline: 2.2334x; 1.0654x over previous
"""NT-Xent contrastive loss on 8 TRN2 NeuronCores — v4 (triangle + host-reduce).

Host ships q = fp8(16 * z/|z|); the device does ONLY similarity matmuls
and the exp, shipping the exp tiles straight back to DRAM.  All
reductions (row sums, symmetry column sums, positive-pair diagonals)
happen on the host in float64.

Per core (own block = 1024 rows of z = concat(z_i, z_j), rotated so rot
block g = abs block (c+g)%8):

  - zq [512, 5120] fp8: rotated q^T, groups 0..4; group 0 is the own
    block and doubles as the matmul rhs (loaded as two i-halves so the
    first matmuls start after 256 KB).
  - rhs4 [512, 1024] fp8: own block with i-halves swapped for cores>=4
    (data-driven g4 quadrant split — the program stays SPMD-uniform
    while core pairs (c, c+4) compute complementary quadrants of their
    shared pair block).
  - g0 (the diagonal block) computes only the upper triangle at 128-row
    granularity: 3 ragged units instead of 4 full ones; the host
    reconstructs the lower triangle from column sums.
  - total exp work: 4.0625 block-equivalents per core (theoretical
    minimum for this 8-way symmetric decomposition), as 17 psum units
    built from chained fp8-DoubleRow matmuls; one wide ScalarE exp per
    unit (constant scale 2/256) — ScalarE is the bottleneck engine.
  - a few dummy matmuls at kernel start warm the PE out of its cold
    p-state while the input DMAs stream.
  - exp tiles ship as fp8e4 (g0..3; the reductions average out the
    quantization noise) and bf16 (g4, which carries the positive-pair
    diagonals for cores 0-3).

Host: den[r] = rotated row sums + column sums of the shipped tiles
minus exp(2) self-sim; pos from cores 0-3; loss = mean(ln den - ln pos).
"""

import os
import sys

for _p in ("/opt/trn_rl_repo", "/opt/pypackages"):
    if os.path.isdir(_p) and _p not in sys.path:
        sys.path.append(_p)

import numpy as np

B = 4096
D = 512
N2 = 2 * B                  # 8192 rows total
NCORES = 8
RPC = N2 // NCORES          # 1024 rows per core
GW = 1024
NG = 5                      # lhsT groups (g4 computed as half-units)
ZSCALE = 16.0               # fp8 scale for q = 16 * u
TAU_INV = 2.0               # 1 / temperature
SC_MUL = TAU_INV / (ZSCALE * ZSCALE)   # exp scale on raw psum
TILE = 128 * 2048

# g0 upper-triangle units: lists of (m, i_start, width)
G0_UNITS = [
    [(0, 0, 1024), (1, 128, 896)],
    [(2, 256, 768), (3, 384, 640)],
    [(4, 512, 512), (5, 640, 384), (6, 768, 256), (7, 896, 128)],
]
# fp8 output slots: 0..2 g0 ragged units, 3..14 g1..3 pairs (mp 0..3)
UNITS8 = [(i, 0, segs) for i, segs in enumerate(G0_UNITS)] + [
    (3 + (g - 1) * 4 + mp, g, [(2 * mp, 0, 1024), (2 * mp + 1, 0, 1024)])
    for g in (1, 2, 3) for mp in range(4)]

_NC_CACHE = {}


def _build_nc():
    from contextlib import ExitStack

    import concourse.bacc as bacc
    import concourse.mybir as mybir
    import concourse.tile as tile

    f32 = mybir.dt.float32
    bf16 = mybir.dt.bfloat16
    f8 = mybir.dt.float8e4
    AF = mybir.ActivationFunctionType
    DR = mybir.MatmulPerfMode.DoubleRow

    nc = bacc.Bacc("TRN2", target_bir_lowering=False, debug=False,
                   num_devices=NCORES)
    zq_dram = nc.dram_tensor("zq", [D, NG * GW], f8,
                             kind="ExternalInput").ap()
    rhs4_dram = nc.dram_tensor("rhs4", [D, GW], f8,
                               kind="ExternalInput").ap()
    oej8_dram = nc.dram_tensor("oej8", [15 * TILE], f8,
                               kind="ExternalOutput").ap()
    oej16_dram = nc.dram_tensor("oej16", [2 * TILE], bf16,
                                kind="ExternalOutput").ap()

    with ExitStack() as ctx:
        tc = ctx.enter_context(tile.TileContext(nc))
        const = ctx.enter_context(tc.tile_pool(name="const", bufs=1))
        keep = ctx.enter_context(tc.tile_pool(name="keep", bufs=1))
        pej = ctx.enter_context(tc.tile_pool(name="pej", bufs=4))
        pps = ctx.enter_context(tc.tile_pool(name="pps", bufs=2,
                                             space="PSUM"))

        dum = const.tile([1, 1], f32, name="dum", tag="dum")
        dumo = const.tile([1, 1], bf16, name="dumo", tag="dumo")
        cwa = const.tile([128, 2, 128], f8, name="cwa", tag="cwa")
        cwb = const.tile([128, 2, 512], f8, name="cwb", tag="cwb")

        # zq0 as two i-halves so the first matmuls start earlier
        zq0h = [keep.tile([128, 4, 512], f8, name=f"zq0_{h}", tag=f"zq0_{h}")
                for h in range(2)]
        zqt = {g: keep.tile([128, 4, GW], f8, name=f"zqt_{g}",
                            tag=f"zqt_{g}") for g in (1, 2, 3, 4)}
        rhs4t = keep.tile([128, 4, GW], f8, name="rhs4t", tag="rhs4t")
        ej4 = [keep.tile([128, 2048], bf16, name=f"ej4_{u}", tag=f"ej4_{u}")
               for u in range(2)]

        # Preload the Exp activation table while input DMAs stream.
        nc.vector.memset(dum[:], 0.0)
        nc.scalar.activation(out=dumo[:], in_=dum[:], func=AF.Exp)
        # PE p-state warmup on const data during the input DMA wait.
        nc.vector.memset(cwa[:], 0.0)
        nc.gpsimd.memset(cwb[:], 0.0)
        wps = pps.tile([128, 2048], f32, name="wps", tag="ps")
        for r in range(6):
            nc.tensor.matmul(wps[:, 0:512], lhsT=cwa[:, :, :],
                             rhs=cwb[:, :, :], start=True, stop=True,
                             perf_mode=DR)

        for h in range(2):
            nc.sync.dma_start(
                out=zq0h[h][:],
                in_=zq_dram[:, h * 512:(h + 1) * 512]
                .rearrange("(j p) n -> p j n", p=128))
        nc.sync.dma_start(
            out=zqt[4][:],
            in_=zq_dram[:, 4 * GW:5 * GW].rearrange("(j p) n -> p j n",
                                                    p=128))
        nc.sync.dma_start(
            out=rhs4t[:],
            in_=rhs4_dram.rearrange("(j p) n -> p j n", p=128))
        for g in (1, 2, 3):
            nc.sync.dma_start(
                out=zqt[g][:],
                in_=zq_dram[:, g * GW:(g + 1) * GW]
                .rearrange("(j p) n -> p j n", p=128))

        def unit8(slot, g, segs, h_first=False):
            """Ragged unit: segs = [(m, i0, w)...]; exp -> fp8 slot."""
            W = sum(w for _, _, w in segs)
            ps = pps.tile([128, 2048], f32, name=f"ps_{slot}", tag="ps")
            chunks = []
            off = 0
            for (m, i0, w) in segs:
                o, i = off, i0
                while w > 0:
                    cw = min(512 - i % 512, w)
                    chunks.append((o, m, i, cw))
                    o += cw
                    i += cw
                    w -= cw
                off = o
            if h_first:               # first unit: zq0 half-0 chunks first
                chunks.sort(key=lambda ch: ch[2] // 512)
            for (o, m, i, cw) in chunks:
                h, ir = i // 512, i % 512
                for kp in range(2):
                    if g == 0:
                        lhsT = zq0h[m // 4][:, 2 * kp:2 * kp + 2,
                                            (m % 4) * 128:(m % 4 + 1) * 128]
                    else:
                        lhsT = zqt[g][:, 2 * kp:2 * kp + 2,
                                      m * 128:(m + 1) * 128]
                    nc.tensor.matmul(
                        ps[:, o:o + cw],
                        lhsT=lhsT,
                        rhs=zq0h[h][:, 2 * kp:2 * kp + 2, ir:ir + cw],
                        start=(kp == 0), stop=(kp == 1), perf_mode=DR)
            ej = pej.tile([128, 2048], f8, name=f"ej_{slot}", tag="ej")
            nc.scalar.activation(out=ej[:, 0:W], in_=ps[:, 0:W],
                                 func=AF.Exp, scale=SC_MUL)
            nc.gpsimd.dma_start(
                out=oej8_dram[slot * TILE:slot * TILE + 128 * W]
                .rearrange("(p n) -> p n", p=128), in_=ej[:, 0:W])

        def unit_g4(u):
            """[128, 4, 512] psum: m-blocks (4u..4u+3) x own i-half."""
            ps = pps.tile([128, 2048], f32, name=f"ps4_{u}", tag="ps")
            for seg in range(4):
                m = u * 4 + seg
                for kp in range(2):
                    nc.tensor.matmul(
                        ps[:, seg * 512:(seg + 1) * 512],
                        lhsT=zqt[4][:, 2 * kp:2 * kp + 2,
                                    m * 128:(m + 1) * 128],
                        rhs=rhs4t[:, 2 * kp:2 * kp + 2,
                                  u * 512:(u + 1) * 512],
                        start=(kp == 0), stop=(kp == 1), perf_mode=DR)
            nc.scalar.activation(out=ej4[u][:], in_=ps[:], func=AF.Exp,
                                 scale=SC_MUL)
            nc.gpsimd.dma_start(
                out=oej16_dram[u * TILE:(u + 1) * TILE]
                .rearrange("(p n) -> p n", p=128), in_=ej4[u][:])

        # schedule: big g0 unit first (needs only zq0), small ones last
        unit8(0, 0, G0_UNITS[0], h_first=True)
        unit_g4(0)
        unit_g4(1)
        for g in (1, 2, 3):
            for mp in range(4):
                slot = 3 + (g - 1) * 4 + mp
                unit8(slot, g, [(2 * mp, 0, 1024), (2 * mp + 1, 0, 1024)])
        unit8(1, 0, G0_UNITS[1])
        unit8(2, 0, G0_UNITS[2])

    nc.compile()
    return nc


def _get_nc():
    if "nc" not in _NC_CACHE:
        _NC_CACHE["nc"] = _build_nc()
    return _NC_CACHE["nc"]


def _in_maps(z):
    import ml_dtypes
    u = z / np.linalg.norm(z, axis=1, keepdims=True)
    q = (ZSCALE * u).astype(ml_dtypes.float8_e4m3)        # [N2, D]
    qT = np.ascontiguousarray(q.T)                        # [D, N2]
    qT2 = np.concatenate([qT, qT[:, :NG * GW]], axis=1)
    maps = []
    for c in range(NCORES):
        zq = np.ascontiguousarray(qT2[:, RPC * c:RPC * c + NG * GW])
        own = qT[:, RPC * c:RPC * (c + 1)]
        if c < 4:
            rhs4 = np.ascontiguousarray(own)
        else:
            rhs4 = np.ascontiguousarray(
                np.concatenate([own[:, 512:], own[:, :512]], axis=1))
        maps.append({"zq": zq, "rhs4": rhs4})
    return maps


def _post(oej8s, oej16s):
    """Host reduction of the shipped exp tiles into the scalar loss."""
    den = np.zeros(N2, np.float64)
    pos = np.zeros(N2, np.float64)
    p128 = np.arange(128)
    for c in range(NCORES):
        t8 = np.asarray(oej8s[c], np.float32)
        iabs = RPC * c + np.arange(GW)
        for (slot, g, segs) in UNITS8:
            W = sum(w for _, _, w in segs)
            arr = t8[slot * TILE:slot * TILE + 128 * W].reshape(128, W)
            off = 0
            for (m, i0, w) in segs:
                T = arr[:, off:off + w]
                off += w
                jabs = (RPC * c + GW * g + m * 128 + p128) % N2
                den[jabs] += T.sum(axis=1, dtype=np.float64)
                if g == 0:
                    den[RPC * c + i0 + 128 + np.arange(w - 128)] += \
                        T[:, 128:].sum(axis=0, dtype=np.float64)
                else:
                    den[RPC * c + i0 + np.arange(w)] += \
                        T.sum(axis=0, dtype=np.float64)
        t16 = np.asarray(oej16s[c], np.float32).reshape(2, 128, 4, 512)
        for uu in range(2):
            T = t16[uu]                            # [128 p, 4 seg, 512 iw]
            rows = T.sum(axis=2, dtype=np.float64)       # [p, seg]
            jabs = (RPC * c + 4096 + uu * 512 + np.arange(512)) % N2
            den[jabs] += rows.T.reshape(512)       # j = seg*128 + p
            cols = T.sum(axis=(0, 1), dtype=np.float64)  # [512]
            off = uu * 512 if c < 4 else (1 - uu) * 512
            den[RPC * c + off + np.arange(512)] += cols
            if c < 4:
                for seg in range(4):
                    x = RPC * c + uu * 512 + seg * 128 + p128
                    d = T[p128, seg, seg * 128 + p128]
                    pos[x] = d
                    pos[x + 4096] = d
    den -= np.exp(TAU_INV)
    rows = np.log(den) - np.log(pos)
    return np.float32(np.mean(rows))


def kernel(z_i: np.ndarray, z_j: np.ndarray) -> np.ndarray:
    from concourse.bass_interp import get_hw_module
    from concourse.bass_utils import run_bass_kernel_spmd

    z = np.concatenate([np.asarray(z_i, np.float32),
                        np.asarray(z_j, np.float32)], axis=0)
    nc = _get_nc()
    old_m = nc.m
    nc.m = get_hw_module(nc.m)
    try:
        res = run_bass_kernel_spmd(nc, _in_maps(z),
                                   core_ids=list(range(NCORES)))
    finally:
        nc.m = old_m

    return _post([res.results[c]["oej8"] for c in range(NCORES)],
                 [res.results[c]["oej16"] for c in range(NCORES)])


# revision 10
# speedup vs baseline: 2.2799x; 1.0208x over previous
"""NT-Xent contrastive loss on 8 TRN2 NeuronCores — v4 (triangle + host-reduce).

Host ships q = fp8(16 * z/|z|); the device does ONLY similarity matmuls
and the exp, shipping the exp tiles straight back to DRAM.  All
reductions (row sums, symmetry column sums, positive-pair diagonals)
happen on the host in float64.

Per core (own block = 1024 rows of z = concat(z_i, z_j), rotated so rot
block g = abs block (c+g)%8):

  - zq [512, 5120] fp8: rotated q^T, groups 0..4; group 0 is the own
    block and doubles as the matmul rhs (loaded as two i-halves so the
    first matmuls start after 256 KB).
  - rhs4 [512, 1024] fp8: own block with i-halves swapped for cores>=4
    (data-driven g4 quadrant split — the program stays SPMD-uniform
    while core pairs (c, c+4) compute complementary quadrants of their
    shared pair block).
  - g0 (the diagonal block) computes only the upper triangle at 128-row
    granularity: 3 ragged units instead of 4 full ones; the host
    reconstructs the lower triangle from column sums.
  - total exp work: 4.0625 block-equivalents per core (theoretical
    minimum for this 8-way symmetric decomposition), as 17 psum units
    built from chained fp8-DoubleRow matmuls; one wide ScalarE exp per
    unit (constant scale 2/256) — ScalarE is the bottleneck engine.
  - a few dummy matmuls at kernel start warm the PE out of its cold
    p-state while the input DMAs stream.
  - exp tiles ship as fp8e4 (g0..3; the reductions average out the
    quantization noise) and bf16 (g4, which carries the positive-pair
    diagonals for cores 0-3).

Host: den[r] = rotated row sums + column sums of the shipped tiles
minus exp(2) self-sim; pos from cores 0-3; loss = mean(ln den - ln pos).
"""

import os
import sys

for _p in ("/opt/trn_rl_repo", "/opt/pypackages"):
    if os.path.isdir(_p) and _p not in sys.path:
        sys.path.append(_p)

import numpy as np

B = 4096
D = 512
N2 = 2 * B                  # 8192 rows total
NCORES = 8
RPC = N2 // NCORES          # 1024 rows per core
GW = 1024
NG = 5                      # lhsT groups (g4 computed as half-units)
ZSCALE = 16.0               # fp8 scale for q = 16 * u
TAU_INV = 2.0               # 1 / temperature
SC_MUL = TAU_INV / (ZSCALE * ZSCALE)   # exp scale on raw psum
TILE = 128 * 2048

# g0 upper-triangle units: lists of (m, i_start, width)
G0_UNITS = [
    [(0, 0, 1024), (1, 128, 896)],
    [(2, 256, 768), (3, 384, 640)],
    [(4, 512, 512), (5, 640, 384), (6, 768, 256), (7, 896, 128)],
]
# fp8 output slots: 0..2 g0 ragged units, 3..14 g1..3 pairs (mp 0..3)
UNITS8 = [(i, 0, segs) for i, segs in enumerate(G0_UNITS)] + [
    (3 + (g - 1) * 4 + mp, g, [(2 * mp, 0, 1024), (2 * mp + 1, 0, 1024)])
    for g in (1, 2, 3) for mp in range(4)]

_NC_CACHE = {}


def _build_nc():
    from contextlib import ExitStack

    import concourse.bacc as bacc
    import concourse.mybir as mybir
    import concourse.tile as tile

    f32 = mybir.dt.float32
    bf16 = mybir.dt.bfloat16
    f8 = mybir.dt.float8e4
    AF = mybir.ActivationFunctionType
    DR = mybir.MatmulPerfMode.DoubleRow

    nc = bacc.Bacc("TRN2", target_bir_lowering=False, debug=False,
                   num_devices=NCORES)
    zq_dram = nc.dram_tensor("zq", [D, NG * GW], f8,
                             kind="ExternalInput").ap()
    rhs4_dram = nc.dram_tensor("rhs4", [D, GW], f8,
                               kind="ExternalInput").ap()
    oej8_dram = nc.dram_tensor("oej8", [15 * TILE], f8,
                               kind="ExternalOutput").ap()
    oej16_dram = nc.dram_tensor("oej16", [2 * TILE], bf16,
                                kind="ExternalOutput").ap()

    with ExitStack() as ctx:
        tc = ctx.enter_context(tile.TileContext(nc))
        const = ctx.enter_context(tc.tile_pool(name="const", bufs=1))
        keep = ctx.enter_context(tc.tile_pool(name="keep", bufs=1))
        pej = ctx.enter_context(tc.tile_pool(name="pej", bufs=4))
        pps = ctx.enter_context(tc.tile_pool(name="pps", bufs=2,
                                             space="PSUM"))

        dum = const.tile([1, 1], f32, name="dum", tag="dum")
        dumo = const.tile([1, 1], bf16, name="dumo", tag="dumo")
        cwa = const.tile([128, 2, 128], f8, name="cwa", tag="cwa")
        cwb = const.tile([128, 2, 512], f8, name="cwb", tag="cwb")

        # zq0 as two i-halves so the first matmuls start earlier
        zq0h = [keep.tile([128, 4, 512], f8, name=f"zq0_{h}", tag=f"zq0_{h}")
                for h in range(2)]
        zqt = {g: keep.tile([128, 4, GW], f8, name=f"zqt_{g}",
                            tag=f"zqt_{g}") for g in (1, 2, 3, 4)}
        rhs4t = keep.tile([128, 4, GW], f8, name="rhs4t", tag="rhs4t")
        ej4 = [keep.tile([128, 2048], bf16, name=f"ej4_{u}", tag=f"ej4_{u}")
               for u in range(2)]

        # Preload the Exp activation table while input DMAs stream.
        nc.vector.memset(dum[:], 0.0)
        nc.scalar.activation(out=dumo[:], in_=dum[:], func=AF.Exp)
        # PE p-state warmup on const data during the input DMA wait.
        nc.vector.memset(cwa[:], 0.0)
        nc.gpsimd.memset(cwb[:], 0.0)
        wps = pps.tile([128, 2048], f32, name="wps", tag="ps")
        for r in range(6):
            o = (r % 2) * 512
            nc.tensor.matmul(wps[:, o:o + 512], lhsT=cwa[:, :, :],
                             rhs=cwb[:, :, :], start=(r < 2), stop=(r >= 4),
                             perf_mode=DR)
        wrd = const.tile([1, 1], bf16, name="wrd", tag="wrd")
        nc.scalar.activation(out=wrd[:], in_=wps[0:1, 0:1], func=AF.Exp)

        for h in range(2):
            nc.sync.dma_start(
                out=zq0h[h][:],
                in_=zq_dram[:, h * 512:(h + 1) * 512]
                .rearrange("(j p) n -> p j n", p=128))
        nc.sync.dma_start(
            out=zqt[1][:],
            in_=zq_dram[:, GW:2 * GW].rearrange("(j p) n -> p j n", p=128))
        nc.sync.dma_start(
            out=zqt[4][:],
            in_=zq_dram[:, 4 * GW:5 * GW].rearrange("(j p) n -> p j n",
                                                    p=128))
        nc.sync.dma_start(
            out=rhs4t[:],
            in_=rhs4_dram.rearrange("(j p) n -> p j n", p=128))
        for g in (2, 3):
            nc.sync.dma_start(
                out=zqt[g][:],
                in_=zq_dram[:, g * GW:(g + 1) * GW]
                .rearrange("(j p) n -> p j n", p=128))

        def unit8(slot, g, segs, h_first=False):
            """Ragged unit: segs = [(m, i0, w)...]; exp -> fp8 slot."""
            W = sum(w for _, _, w in segs)
            ps = pps.tile([128, 2048], f32, name=f"ps_{slot}", tag="ps")
            chunks = []
            off = 0
            for (m, i0, w) in segs:
                o, i = off, i0
                while w > 0:
                    cw = min(512 - i % 512, w)
                    chunks.append((o, m, i, cw))
                    o += cw
                    i += cw
                    w -= cw
                off = o
            if h_first:               # first unit: zq0 half-0 chunks first
                chunks.sort(key=lambda ch: ch[2] // 512)
            for (o, m, i, cw) in chunks:
                h, ir = i // 512, i % 512
                for kp in range(2):
                    if g == 0:
                        lhsT = zq0h[m // 4][:, 2 * kp:2 * kp + 2,
                                            (m % 4) * 128:(m % 4 + 1) * 128]
                    else:
                        lhsT = zqt[g][:, 2 * kp:2 * kp + 2,
                                      m * 128:(m + 1) * 128]
                    nc.tensor.matmul(
                        ps[:, o:o + cw],
                        lhsT=lhsT,
                        rhs=zq0h[h][:, 2 * kp:2 * kp + 2, ir:ir + cw],
                        start=(kp == 0), stop=(kp == 1), perf_mode=DR)
            ej = pej.tile([128, 2048], f8, name=f"ej_{slot}", tag="ej")
            nc.scalar.activation(out=ej[:, 0:W], in_=ps[:, 0:W],
                                 func=AF.Exp, scale=SC_MUL)
            nc.gpsimd.dma_start(
                out=oej8_dram[slot * TILE:slot * TILE + 128 * W]
                .rearrange("(p n) -> p n", p=128), in_=ej[:, 0:W])

        def unit_g4(u):
            """[128, 4, 512] psum: m-blocks (4u..4u+3) x own i-half."""
            ps = pps.tile([128, 2048], f32, name=f"ps4_{u}", tag="ps")
            for seg in range(4):
                m = u * 4 + seg
                for kp in range(2):
                    nc.tensor.matmul(
                        ps[:, seg * 512:(seg + 1) * 512],
                        lhsT=zqt[4][:, 2 * kp:2 * kp + 2,
                                    m * 128:(m + 1) * 128],
                        rhs=rhs4t[:, 2 * kp:2 * kp + 2,
                                  u * 512:(u + 1) * 512],
                        start=(kp == 0), stop=(kp == 1), perf_mode=DR)
            nc.scalar.activation(out=ej4[u][:], in_=ps[:], func=AF.Exp,
                                 scale=SC_MUL)
            nc.gpsimd.dma_start(
                out=oej16_dram[u * TILE:(u + 1) * TILE]
                .rearrange("(p n) -> p n", p=128), in_=ej4[u][:])

        # schedule follows DMA arrival order; small g0 units last
        unit8(0, 0, G0_UNITS[0], h_first=True)
        for mp in range(4):
            unit8(3 + mp, 1, [(2 * mp, 0, 1024), (2 * mp + 1, 0, 1024)])
        unit_g4(0)
        unit_g4(1)
        for g in (2, 3):
            for mp in range(4):
                slot = 3 + (g - 1) * 4 + mp
                unit8(slot, g, [(2 * mp, 0, 1024), (2 * mp + 1, 0, 1024)])
        unit8(1, 0, G0_UNITS[1])
        unit8(2, 0, G0_UNITS[2])

    nc.compile()
    return nc


def _get_nc():
    if "nc" not in _NC_CACHE:
        _NC_CACHE["nc"] = _build_nc()
    return _NC_CACHE["nc"]


def _in_maps(z):
    import ml_dtypes
    u = z / np.linalg.norm(z, axis=1, keepdims=True)
    q = (ZSCALE * u).astype(ml_dtypes.float8_e4m3)        # [N2, D]
    qT = np.ascontiguousarray(q.T)                        # [D, N2]
    qT2 = np.concatenate([qT, qT[:, :NG * GW]], axis=1)
    maps = []
    for c in range(NCORES):
        zq = np.ascontiguousarray(qT2[:, RPC * c:RPC * c + NG * GW])
        own = qT[:, RPC * c:RPC * (c + 1)]
        if c < 4:
            rhs4 = np.ascontiguousarray(own)
        else:
            rhs4 = np.ascontiguousarray(
                np.concatenate([own[:, 512:], own[:, :512]], axis=1))
        maps.append({"zq": zq, "rhs4": rhs4})
    return maps


def _post(oej8s, oej16s):
    """Host reduction of the shipped exp tiles into the scalar loss."""
    den = np.zeros(N2, np.float64)
    pos = np.zeros(N2, np.float64)
    p128 = np.arange(128)
    for c in range(NCORES):
        t8 = np.asarray(oej8s[c], np.float32)
        iabs = RPC * c + np.arange(GW)
        for (slot, g, segs) in UNITS8:
            W = sum(w for _, _, w in segs)
            arr = t8[slot * TILE:slot * TILE + 128 * W].reshape(128, W)
            off = 0
            for (m, i0, w) in segs:
                T = arr[:, off:off + w]
                off += w
                jabs = (RPC * c + GW * g + m * 128 + p128) % N2
                den[jabs] += T.sum(axis=1, dtype=np.float64)
                if g == 0:
                    den[RPC * c + i0 + 128 + np.arange(w - 128)] += \
                        T[:, 128:].sum(axis=0, dtype=np.float64)
                else:
                    den[RPC * c + i0 + np.arange(w)] += \
                        T.sum(axis=0, dtype=np.float64)
        t16 = np.asarray(oej16s[c], np.float32).reshape(2, 128, 4, 512)
        for uu in range(2):
            T = t16[uu]                            # [128 p, 4 seg, 512 iw]
            rows = T.sum(axis=2, dtype=np.float64)       # [p, seg]
            jabs = (RPC * c + 4096 + uu * 512 + np.arange(512)) % N2
            den[jabs] += rows.T.reshape(512)       # j = seg*128 + p
            cols = T.sum(axis=(0, 1), dtype=np.float64)  # [512]
            off = uu * 512 if c < 4 else (1 - uu) * 512
            den[RPC * c + off + np.arange(512)] += cols
            if c < 4:
                for seg in range(4):
                    x = RPC * c + uu * 512 + seg * 128 + p128
                    d = T[p128, seg, seg * 128 + p128]
                    pos[x] = d
                    pos[x + 4096] = d
    den -= np.exp(TAU_INV)
    rows = np.log(den) - np.log(pos)
    return np.float32(np.mean(rows))


def kernel(z_i: np.ndarray, z_j: np.ndarray) -> np.ndarray:
    from concourse.bass_interp import get_hw_module
    from concourse.bass_utils import run_bass_kernel_spmd

    z = np.concatenate([np.asarray(z_i, np.float32),
                        np.asarray(z_j, np.float32)], axis=0)
    nc = _get_nc()
    old_m = nc.m
    nc.m = get_hw_module(nc.m)
    try:
        res = run_bass_kernel_spmd(nc, _in_maps(z),
                                   core_ids=list(range(NCORES)))
    finally:
        nc.m = old_m

    return _post([res.results[c]["oej8"] for c in range(NCORES)],
                 [res.results[c]["oej16"] for c in range(NCORES)])
